# revision 8
# baseline (speedup 1.0000x reference)
"""Trainium2 Bass kernel for nn_ColorHistograms (v3).

Per NeuronCore (2 batch elements, 8 cores):
  x is quantized to uint8 on the host (rel-err ~2e-3 << 2e-2 gate): 8MB/core
  HBM instead of 32MB f32. The SDMA engines price a cast-DMA at its WRITE
  side, so only 3 of the 6 (batch, channel) planes stream via SWDGE
  u8->fp16 cast in [s-partition, t] layout for the PE; the other 3 stream
  as plain u8 [frame, s] tiles reduced by DVE/ScalarE at 1 elem/cycle.
    cast (PE means): (b0,c2) (b1,c2) (b1,c1) -- PE LDWEIGHTS(chunk [128s x
      128t]) + 1-col ones matmul, ~27-53ns per 16K elements.
    plain (DVE/ACT): (b0,c0) (b1,c0) (b0,c1).
  Engine-side DMA bytes: 3*2.65MB u8 + 3*5.3MB cast-write + gathers/outs
  ~ 14.4MB at ~335 B/ns.
  Means -> PE transpose -> scale+center -> fp16 -> staged to DRAM in NREP
  replicas (Toeplitz gather engine spread); staging/consts on the HWDGE
  rings so they never queue behind cast DMAs. Toeplitz gather builds 101
  shifted copies [101, 3c x 1024t]; base row broadcast via PE ones-matmul
  into PSUM; DVE subtract + abs-channel-reduce + mask; fp16 matmuls with
  fc weights (bias as extra contraction row) + ReLU on ScalarE + store.
"""

import sys

if "/opt/trn_rl_repo" not in sys.path:
    sys.path.insert(0, "/opt/trn_rl_repo")

import numpy as np

N_CORES = 8
B, T, H, W, C = 16, 1024, 27, 48, 3
S = H * W
LW = 101
PAD = 50
OD = 128
BPC = B // N_CORES
PADROW = T + LW - 1       # 1124
FT = T // 128
SCALE = 1.0 / (255.0 * S)
CENTER = 0.5
SJ = 10                   # full 128-row s-chunks; tail chunk has 16 rows
STAIL = S - SJ * 128
NREP = 8
REPS = 70656              # fp16 elements between mean-scratch replicas
GCH = [(0, 13), (13, 26), (26, 39), (39, 52), (52, 65), (65, 78),
       (78, 91), (91, 101)]
# cast slabs (PE): sl -> (batch, channel); plain planes (DVE/ACT)
SLABS = [(0, 2), (1, 2), (1, 1)]
PLAINS = [(0, 0), (1, 0), (0, 1)]

_CACHE = {}


def _build_program():
    import concourse.bass as bass
    import concourse.tile as tile
    from concourse import bacc, mybir
    from concourse.ap import AP

    f32 = mybir.dt.float32
    f16 = mybir.dt.float16
    bf16 = mybir.dt.bfloat16
    u8 = mybir.dt.uint8
    nc = bacc.Bacc("TRN2", target_bir_lowering=False, debug=False)

    # plain planes: rows = plane*T + t
    xp = nc.dram_tensor("xp", [len(PLAINS) * T, S], u8, kind="ExternalInput")
    # cast slabs: rows = sl*128 + p, cols = j*T + t  (s = j*128 + p)
    x12 = nc.dram_tensor("x12", [len(SLABS) * 128, SJ * T], u8,
                         kind="ExternalInput")
    x12t = nc.dram_tensor("x12t", [len(SLABS) * STAIL, T], u8,
                          kind="ExternalInput")
    fcwb = nc.dram_tensor("fcwb", [LW + 1, OD], f16, kind="ExternalInput")
    maskw = nc.dram_tensor("maskw", [LW, T], bf16, kind="ExternalInput")
    ident = nc.dram_tensor("ident", [128, 128], f32, kind="ExternalInput")
    y = nc.dram_tensor("y", [BPC * T, OD], f32, kind="ExternalOutput")
    mcpad = nc.dram_tensor("mcpad", [NREP * REPS], f16)
    mc_ap = mcpad[:]

    def mc_view(offset, dims):
        return AP(tensor=mc_ap.tensor, offset=offset, ap=tuple(dims))

    with tile.TileContext(nc) as tc:
        with (
            tc.tile_pool(name="consts", bufs=1) as consts,
            tc.tile_pool(name="xpin", bufs=10) as xpin,
            tc.tile_pool(name="cast", bufs=3) as castp,
            tc.tile_pool(name="ctail", bufs=3) as ctailp,
            tc.tile_pool(name="sums", bufs=2) as sumsp,
            tc.tile_pool(name="stg", bufs=2) as stgp,
            tc.tile_pool(name="mrow", bufs=2) as mrowp,
            tc.tile_pool(name="gath", bufs=2) as gathp,
            tc.tile_pool(name="wf", bufs=2) as wfp,
            tc.tile_pool(name="outs", bufs=4) as outsp,
            tc.tile_pool(name="junk", bufs=1) as junkp,
            tc.tile_pool(name="zrow", bufs=1) as zrowp,
            tc.tile_pool(name="pmean", bufs=2, space="PSUM") as pmean,
            tc.tile_pool(name="pba", bufs=2, space="PSUM") as pba,
            tc.tile_pool(name="ppt", bufs=1, space="PSUM") as ppt,
            tc.tile_pool(name="pout", bufs=2, space="PSUM") as pout,
        ):
            fcwb_sb = consts.tile([LW + 1, OD], f16)
            nc.sync.dma_start(fcwb_sb[:], fcwb[:])
            maskw_sb = consts.tile([LW, T], bf16)
            nc.scalar.dma_start(maskw_sb[:], maskw[:])
            ident_sb = consts.tile([128, 128], f32)
            nc.sync.dma_start(ident_sb[:], ident[:])
            ones_sb = consts.tile([128, 1], f16)
            nc.vector.memset(ones_sb[:], 1.0)
            onesw_sb = consts.tile([1, LW], f16)
            nc.vector.memset(onesw_sb[:], 1.0)

            z = zrowp.tile([48, PADROW], f16)
            nc.vector.memset(z[:], 0.0)
            nc.scalar.dma_start(
                mc_view(0, [(REPS, NREP), (1, BPC * C * PADROW)]), z[:]
            )

            # ---- x stream: all DMAs up front; b0 data first ----
            xpt, cst, cstl = {}, {}, {}

            def emit_plain(pi):
                for i in range(FT):
                    xt = xpin.tile([128, S], u8, name="xpt")
                    xpt[(pi, i)] = xt
                    eng = nc.sync if i % 2 == 0 else nc.scalar
                    eng.dma_start(
                        xt[:], xp[pi * T + i * 128: pi * T + (i + 1) * 128, :])

            def emit_cast(sl):
                ct = castp.tile([128, SJ * T], f16, name="cslab")
                cst[sl] = ct
                ctv = ct[:].rearrange("p (j t) -> p j t", j=SJ)
                row0 = sl * 128
                for h in range(2):          # t-halves
                    for g in range(2):      # j-halves
                        nc.gpsimd.dma_start(
                            ctv[:, g * 5:(g + 1) * 5, h * 512:(h + 1) * 512],
                            AP(tensor=x12[:].tensor,
                               offset=row0 * SJ * T + g * 5 * T + h * 512,
                               ap=((SJ * T, 128), (T, 5), (1, 512))),
                        )
                ctl = ctailp.tile([STAIL, T], f16, name="ctail")
                cstl[sl] = ctl
                nc.gpsimd.dma_start(
                    ctl[:], x12t[sl * STAIL: (sl + 1) * STAIL, :])

            # b0 first: plain (0,c0), (0,c1) and cast slab 0 = (0,c2)
            emit_plain(0)      # (0, c0)
            emit_cast(0)       # (0, c2)
            emit_plain(2)      # (0, c1)
            # b1: cast first (PE latency), then plain
            emit_cast(1)       # (1, c2)
            emit_cast(2)       # (1, c1)
            emit_plain(1)      # (1, c0)

            sums_t, sh_t, mrow_t = {}, {}, {}

            def plain_means(pi, i):
                b, c = PLAINS[pi]
                sums = sums_t[b]
                xt = xpt[(pi, i)]
                col = c * FT + i
                if i % 3 == 0:
                    nc.vector.tensor_reduce(
                        sums[:, col:col + 1], xt[:],
                        axis=mybir.AxisListType.X, op=mybir.AluOpType.add)
                else:
                    jk = junkp.tile([128, S], f32)
                    nc.scalar.activation(
                        jk[:], xt[:], mybir.ActivationFunctionType.Copy,
                        bias=0.0, scale=1.0, accum_out=sums[:, col:col + 1])

            def pe_means(sl, half):
                b, c = SLABS[sl]
                ct = cst[sl]
                ctv = ct[:].rearrange("p (j t) -> p j t", j=SJ)
                ctl = cstl[sl]
                pm = pmean_t[b]
                for jt in range(4 * half, 4 * (half + 1)):
                    col = c * FT + jt
                    for j in range(SJ):
                        nc.tensor.matmul(
                            pm[:, col:col + 1],
                            ctv[:, j, jt * 128:(jt + 1) * 128],
                            ones_sb[:],
                            start=(j == 0), stop=False)
                    nc.tensor.matmul(
                        pm[:, col:col + 1],
                        ctl[:, jt * 128:(jt + 1) * 128],
                        ones_sb[0:STAIL, :],
                        start=False, stop=True)

            def finish_means(b):
                sums = sums_t[b]
                pm = pmean_t[b]
                pe_cols = sorted(c * FT for (bb, c) in SLABS if bb == b)
                lo, hi = pe_cols[0], pe_cols[-1] + FT
                nc.vector.tensor_copy(sums[:, lo:hi], pm[:, lo:hi])
                pt = ppt.tile([C * FT, 128], f32, name="pt")
                nc.tensor.transpose(pt[:], sums[:], ident_sb[:])
                stg = stgp.tile([C * FT, NREP * 128], f16, name="stg")
                nc.vector.tensor_scalar(
                    out=stg[:, 0:128], in0=pt[:], scalar1=SCALE,
                    scalar2=CENTER, op0=mybir.AluOpType.mult,
                    op1=mybir.AluOpType.subtract)
                for d in (128, 256, 512):
                    nc.vector.tensor_copy(stg[:, d:2 * d], stg[:, 0:d])
                for c in range(C):
                    eng = nc.sync if c % 2 == 0 else nc.scalar
                    eng.dma_start(
                        mc_view(b * C * PADROW + c * PADROW + PAD,
                                [(128, FT), (REPS, NREP), (1, 128)]),
                        stg[c * FT:(c + 1) * FT, :].rearrange(
                            "p (r t) -> p r t", r=NREP),
                    )
                mrow = mrowp.tile([1, C * T], f16, name="mrow")
                mrow_t[b] = mrow
                nc.scalar.dma_start(
                    mrow[:],
                    mc_view(b * C * PADROW + PAD,
                            [(1, 1), (PADROW, C), (1, T)]),
                )

            def gather(b):
                sh = gathp.tile([LW, C * T], f16, name="sh", tag="sh")
                sh_t[b] = sh
                for k, (w0, w1) in enumerate(GCH):
                    eng = nc.scalar if k % 2 == 0 else nc.sync
                    eng.dma_start(
                        sh[w0:w1, :],
                        mc_view(k * REPS + b * C * PADROW + w0,
                                [(1, w1 - w0), (PADROW, C), (1, T)]),
                    )

            def dist(b):
                sh = sh_t[b]
                mrow = mrow_t[b]
                wf = wfp.tile([LW + 1, T], f16, name="wf")
                nc.vector.memset(wf[:], 1.0)
                for c in range(C):
                    for hh in range(2):
                        lo = c * T + hh * 512
                        bap = pba.tile([LW, 512], f32, name="ba")
                        nc.tensor.matmul(
                            bap[:], onesw_sb[:], mrow[:, lo:lo + 512],
                            start=True, stop=True)
                        nc.vector.tensor_sub(
                            sh[:, lo:lo + 512], sh[:, lo:lo + 512], bap[:])
                with nc.allow_low_precision(reason="3-term abs sum in f16"):
                    nc.vector.tensor_reduce(
                        wf[0:LW, :], sh[:].rearrange("p (c t) -> p t c", c=C),
                        axis=mybir.AxisListType.X, op=mybir.AluOpType.add,
                        apply_absolute_value=True)
                nc.vector.tensor_mul(wf[0:LW, :], wf[0:LW, :], maskw_sb[:])
                return wf

            def matmuls(b, wf):
                for j in range(FT):
                    po = pout.tile([128, OD], f32, name="po")
                    nc.tensor.matmul(po[:], wf[:, bass.ts(j, 128)],
                                     fcwb_sb[:], start=True, stop=True)
                    osb = outsp.tile([128, OD], f32, name="osb")
                    nc.scalar.activation(
                        osb[:], po[:], mybir.ActivationFunctionType.Relu)
                    eng = nc.sync if j % 2 == 0 else nc.scalar
                    eng.dma_start(
                        y[b * T + j * 128: b * T + (j + 1) * 128, :], osb[:])

            # ---- emission schedule ----
            pmean_t = {}
            for b in range(BPC):
                sums_t[b] = sumsp.tile([128, C * FT], f32, name="sums")
                pmean_t[b] = pmean.tile([128, C * FT], f32, name="pm")
            for i in range(FT):
                plain_means(0, i)       # (0, c0)
                plain_means(2, i)       # (0, c1)
            pe_means(0, 0)
            pe_means(0, 1)
            finish_means(0)
            gather(0)
            for i in range(FT):
                plain_means(1, i)       # (1, c0)
            wf0 = dist(0)
            matmuls(0, wf0)
            for sl in (1, 2):
                pe_means(sl, 0)
                pe_means(sl, 1)
            finish_means(1)
            gather(1)
            wf1 = dist(1)
            matmuls(1, wf1)

    nc.compile()
    return nc


def get_nc():
    if "nc" not in _CACHE:
        _CACHE["nc"] = _build_program()
    return _CACHE["nc"]


def make_host_inputs(x, fc_w, fc_b):
    import ml_dtypes

    xq = np.clip(np.rint(np.asarray(x, dtype=np.float32) * 255.0), 0, 255)
    xq = xq.astype(np.uint8).reshape(B, T, S, C)

    wT = fc_w.T.astype(np.float32)
    fcwb = np.concatenate([wT, fc_b[None, :].astype(np.float32)], axis=0)
    fcwb = np.ascontiguousarray(fcwb.astype(np.float16))
    u = np.arange(T)[None, :] + np.arange(LW)[:, None] - PAD
    maskw = np.ascontiguousarray(
        ((u >= 0) & (u < T)).astype(ml_dtypes.bfloat16))
    ident = np.eye(128, dtype=np.float32)
    in_maps = []
    for ci in range(N_CORES):
        bg = ci * BPC
        xpl = np.stack([xq[bg + b, :, :, c] for b, c in PLAINS])  # [3,T,S]
        slabs, tails = [], []
        for b, c in SLABS:
            pl = xq[bg + b, :, :SJ * 128, c]                      # [T, 1280]
            pl = pl.reshape(T, SJ, 128).transpose(2, 1, 0)        # [128,SJ,T]
            slabs.append(pl.reshape(128, SJ * T))
            tails.append(
                np.ascontiguousarray(xq[bg + b, :, SJ * 128:, c].T))
        in_maps.append({
            "xp": np.ascontiguousarray(xpl.reshape(len(PLAINS) * T, S)),
            "x12": np.ascontiguousarray(np.concatenate(slabs, axis=0)),
            "x12t": np.ascontiguousarray(np.concatenate(tails, axis=0)),
            "fcwb": fcwb, "maskw": maskw, "ident": ident,
        })
    return in_maps


def kernel(x, fc_w, fc_b):
    from concourse.bass_utils import run_bass_kernel_spmd

    nc = get_nc()
    in_maps = make_host_inputs(x, fc_w, fc_b)
    res = run_bass_kernel_spmd(nc, in_maps, list(range(N_CORES)))
    outs = [r["y"].reshape(BPC, T, OD) for r in res.results]
    return np.concatenate(outs, axis=0).astype(np.float32)
